# revision 6
# baseline (speedup 1.0000x reference)
"""Trainium2 Bass kernel for BiDAF-style bidirectional attention.

Reference math (per batch b):
    sim[c,q]  = q[q]·wq + c[c]·wc + sum_e wm[e]*question[q,e]*context[c,e]
    c2q[c,:]  = softmax_q(sim[c,:]) @ question          # (C, E)
    q2c[:]    = softmax_c(max_q sim[c,:]) @ context     # (E,)
    out[c,:]  = [context | c2q | context*c2q | context*q2c]

Sharding: pure data parallel over batch (B=16 -> 2 batches per core x 8 cores).

Restructured pipeline (vs. the first working version):
  - ALL DMA loads (params, question, every context group) are emitted up
    front so no load ever queues behind a semaphore-blocked store on the
    sync HWDGE ring.
  - the q_weighted row is folded into the sim matmul as a K=1 rank-1
    update (ones ⊗ qw_pad), removing the per-pair DVE add + broadcast.
  - per-pair (2 context tiles) granularity everywhere: one PE-transpose
    bank, one PSUM->SBUF copy, one 3D reduce_max, one bf16 pt copy.
  - exp reads sim straight out of PSUM (bias = -rowmax) and emits bf16
    attention weights + f32 row sums.
  - per-batch epilogue (q2c softmax over C + context*q2c + final column
    stores) is emitted immediately after that batch's pass 1, so batch
    0's tail writes overlap batch 1's compute.
  - q2c rank-1 matmuls run on f32r-bitcast views of the staged f32
    context (no bf16 shadow copies of the context at all).
  - outputs stream per pair: cols 0:3E as soon as ctx*c2q is done,
    cols 3E:4E after the batch epilogue.
"""

import numpy as np

import concourse.bass as bass
import concourse.tile as tile
import concourse.mybir as mybir
from concourse import bacc
from concourse.bass_utils import run_bass_kernel_spmd
from concourse.masks import make_identity

B, C, Q, E = 16, 2048, 128, 256
NCORES = 8
BPC = B // NCORES          # batches per core
NT = C // 128              # context tiles per batch
NG = NT // 4               # groups of 4 tiles
F32 = mybir.dt.float32
F32R = mybir.dt.float32r
BF16 = mybir.dt.bfloat16
AX = mybir.AxisListType.X
EXP = mybir.ActivationFunctionType.Exp
CPY = mybir.ActivationFunctionType.Copy


def _body(tc, out_ext, ctx_in, q_in, wq_in, wc_in, wm_in):
    nc = tc.nc
    with (
        tc.tile_pool(name="singles", bufs=1) as singles,
        tc.tile_pool(name="stgp", bufs=BPC * NG) as stgp,
        tc.tile_pool(name="qside", bufs=2) as qside,
        tc.tile_pool(name="xbfp", bufs=BPC * NG) as xbfp,
        tc.tile_pool(name="work", bufs=4) as work,
        tc.tile_pool(name="statsp", bufs=2) as statsp,
        tc.tile_pool(name="ps_xct", bufs=2, space="PSUM") as ps_xct,
        tc.tile_pool(name="ps_sim", bufs=3, space="PSUM") as ps_sim,
        tc.tile_pool(name="ps_pt", bufs=1, space="PSUM") as ps_pt,
        tc.tile_pool(name="ps_c2q", bufs=1, space="PSUM") as ps_c2q,
        tc.tile_pool(name="ps_misc", bufs=1, space="PSUM") as ps_misc,
    ):
        # ---- constants + params ------------------------------------------
        ident = singles.tile([128, 128], F32)
        make_identity(nc, ident)
        ident_bf = singles.tile([128, 128], BF16)
        make_identity(nc, ident_bf)
        ones_r = singles.tile([1, 128], F32)
        nc.vector.memset(ones_r, 1.0)
        ones_c = singles.tile([128, 1], F32)
        nc.vector.memset(ones_c, 1.0)
        ones_m = singles.tile([1, 128], F32R)
        nc.vector.tensor_copy(out=ones_m, in_=ones_r)
        wq_sb = singles.tile([128, 2], F32)
        nc.sync.dma_start(out=wq_sb, in_=wq_in.rearrange("(j p) -> p j", p=128))
        wc_sb = singles.tile([128, 2], F32)
        nc.sync.dma_start(out=wc_sb, in_=wc_in.rearrange("(j p) -> p j", p=128))
        wm_sb = singles.tile([128, 2], F32)
        nc.sync.dma_start(out=wm_sb, in_=wm_in.rearrange("(j p) -> p j", p=128))

        # ---- all loads up front ------------------------------------------
        qms = []
        for b in range(BPC):
            qm = qside.tile([128, E], F32, tag="qm")
            nc.sync.dma_start(out=qm, in_=q_in[b])
            qms.append(qm)
        stgs = {}
        for b in range(BPC):
            stgs[b] = []
            for g in range(NG):
                stg = stgp.tile([128, 4, 4 * E], F32, tag="stg")
                stgs[b].append(stg)
                nc.sync.dma_start(
                    out=stg[:, :, 0:E],
                    in_=ctx_in[b, g * 512 : (g + 1) * 512, :].rearrange(
                        "(t p) e -> p t e", p=128
                    ),
                )

        # ---- phase A: question-side prep for both batches ----------------
        rhs_augs, qw_pads, qm_bfs, mstats = [], [], [], []
        for b in range(BPC):
            qm = qms[b]
            qmt_ps = ps_xct.tile([128, E], F32, tag="xct")
            for j in range(2):
                nc.tensor.transpose(
                    qmt_ps[:, j * 128 : (j + 1) * 128],
                    qm[:, j * 128 : (j + 1) * 128],
                    ident,
                )
            qmt_sb = qside.tile([128, E], F32, tag="qmt")
            nc.vector.tensor_copy(out=qmt_sb, in_=qmt_ps)
            qm_bf = qside.tile([128, E], BF16, tag="qmbf")
            nc.vector.tensor_copy(out=qm_bf, in_=qm)
            qm_bfs.append(qm_bf)
            # rhs_aug[:, j, 0:128] = wm-chunk * QmT-chunk ; [:, j, 128] = wc-chunk
            # cols 129:256 are zero padding so the fp32r matmul runs at N=256.
            rhs_aug = qside.tile([128, 2, E], F32R, tag="rhs_aug")
            for j in range(2):
                nc.vector.tensor_scalar_mul(
                    rhs_aug[:, j, 0:128],
                    qmt_sb[:, j * 128 : (j + 1) * 128],
                    wm_sb[:, j : j + 1],
                )
                nc.vector.tensor_copy(
                    out=rhs_aug[:, j, 128:129], in_=wc_sb[:, j : j + 1]
                )
                nc.vector.tensor_scalar_mul(
                    rhs_aug[:, j, 129:256],
                    qmt_sb[:, j * 128 : (j + 1) * 128][:, 0:127],
                    0.0,
                )
            rhs_augs.append(rhs_aug)
            # q-weighted row, padded to N=256 for the K=1 rank-1 sim update
            qw_ps = ps_misc.tile([1, 128], F32, tag="misc")
            for j in range(2):
                nc.tensor.matmul(
                    qw_ps,
                    wq_sb[:, j : j + 1],
                    qmt_sb[:, j * 128 : (j + 1) * 128],
                    start=(j == 0),
                    stop=(j == 1),
                )
            qw_pad = qside.tile([1, 256], F32R, tag="qw_pad")
            nc.vector.tensor_copy(out=qw_pad[:, 0:128], in_=qw_ps)
            nc.vector.tensor_scalar_mul(qw_pad[:, 128:256], qw_ps, 0.0)
            qw_pads.append(qw_pad)
            mstat = statsp.tile([128, NT], F32, tag="mstat")
            mstats.append(mstat)

        # ---- per batch: pass 1 + immediate epilogue ----------------------
        for b in range(BPC):
            rhs_aug, qw_pad, qm_bf, mstat = (
                rhs_augs[b], qw_pads[b], qm_bfs[b], mstats[b],
            )
            xcbfs = []
            for g in range(NG):
                stg = stgs[b][g]
                # bf16 shadow of the context for the epilogue q2c matmuls
                xcbf = xbfp.tile([128, 4, E], BF16, tag="xcbf")
                nc.vector.tensor_copy(out=xcbf, in_=stg[:, :, 0:E])
                xcbfs.append(xcbf)
                for h in range(2):
                    # ---- sim for both tiles of the pair ------------------
                    xct_ps = ps_xct.tile([128, 2, E], F32, tag="xct")
                    for i in range(2):
                        for j in range(2):
                            nc.tensor.transpose(
                                xct_ps[:, i, j * 128 : (j + 1) * 128],
                                stg[:, 2 * h + i, j * 128 : (j + 1) * 128],
                                ident,
                            )
                    xct_sb = work.tile([128, 2, E], F32R, tag="xct_sb")
                    nc.vector.tensor_copy(out=xct_sb, in_=xct_ps)
                    sim_ps = ps_sim.tile([128, 2, E], F32, tag="sim")
                    for i in range(2):
                        nc.tensor.matmul(
                            sim_ps[:, i, :],
                            xct_sb[:, i, 0:128],
                            rhs_aug[:, 0, :],
                            start=True,
                            stop=False,
                        )
                        nc.tensor.matmul(
                            sim_ps[:, i, :],
                            xct_sb[:, i, 128:256],
                            rhs_aug[:, 1, :],
                            start=False,
                            stop=False,
                        )
                        nc.tensor.matmul(
                            sim_ps[:, i, :],
                            ones_m,
                            qw_pad,
                            start=False,
                            stop=True,
                        )
                    # ---- softmax stats straight from PSUM ----------------
                    neg_m = work.tile([128, 2], F32, tag="neg_m")
                    nc.vector.reduce_max(
                        out=neg_m, in_=sim_ps[:, :, 0:128], axis=AX, negate=True
                    )
                    t0 = 4 * g + 2 * h
                    nc.vector.tensor_sub(
                        mstat[:, t0 : t0 + 2], sim_ps[:, :, 128], neg_m
                    )
                    row_sum = work.tile([128, 2], F32, tag="row_sum")
                    recip = work.tile([128, 2], F32, tag="recip")
                    p_sb = work.tile([128, 2, 128], BF16, tag="p_sb")
                    for i in range(2):
                        nc.scalar.activation(
                            out=p_sb[:, i, :],
                            in_=sim_ps[:, i, 0:128],
                            func=EXP,
                            bias=neg_m[:, i : i + 1],
                            scale=1.0,
                            accum_out=row_sum[:, i : i + 1],
                        )
                    nc.vector.reciprocal(out=recip, in_=row_sum)
                    # ---- c2q attention -----------------------------------
                    pt_ps = ps_pt.tile([128, 2, 128], BF16, tag="pt")
                    for i in range(2):
                        nc.tensor.transpose(
                            pt_ps[:, i, :], p_sb[:, i, :], ident_bf
                        )
                    pt_sb = work.tile([128, 2, 128], BF16, tag="pt_sb")
                    nc.vector.tensor_copy(out=pt_sb, in_=pt_ps)
                    c2q_ps = ps_c2q.tile([128, 2, E], F32, tag="c2q")
                    for i in range(2):
                        nc.tensor.matmul(
                            c2q_ps[:, i, :],
                            pt_sb[:, i, :],
                            qm_bf,
                            start=True,
                            stop=True,
                        )
                    for i in range(2):
                        nc.scalar.activation(
                            out=stg[:, 2 * h + i, E : 2 * E],
                            in_=c2q_ps[:, i, :],
                            func=CPY,
                            scale=recip[:, i : i + 1],
                        )
                    # ---- ctx * c2q + stream out cols 0:3E ----------------
                    nc.gpsimd.tensor_mul(
                        stg[:, 2 * h : 2 * h + 2, 2 * E : 3 * E],
                        stg[:, 2 * h : 2 * h + 2, 0:E],
                        stg[:, 2 * h : 2 * h + 2, E : 2 * E],
                    )
                    r0 = g * 512 + h * 256
                    nc.sync.dma_start(
                        out=out_ext[b, r0 : r0 + 256, 0 : 3 * E].rearrange(
                            "(t p) f -> p t f", p=128
                        ),
                        in_=stg[:, 2 * h : 2 * h + 2, 0 : 3 * E],
                    )

            # ---- epilogue: q2c softmax over C ----------------------------
            r1 = statsp.tile([128, 1], F32, tag="r1")
            nc.vector.reduce_max(out=r1, in_=mstat, axis=AX)
            r1t_ps = ps_misc.tile([1, 128], F32, tag="misc")
            nc.tensor.transpose(r1t_ps, r1, ident)
            neg_gmax = statsp.tile([1, 1], F32, tag="gmax")
            nc.vector.reduce_max(out=neg_gmax, in_=r1t_ps, axis=AX, negate=True)
            ngb_ps = ps_misc.tile([128, 1], F32, tag="misc")
            nc.tensor.matmul(ngb_ps, ones_r, neg_gmax, start=True, stop=True)
            ngb_sb = statsp.tile([128, 1], F32, tag="ngb")
            nc.vector.tensor_copy(out=ngb_sb, in_=ngb_ps)
            e_sb = statsp.tile([128, NT], BF16, tag="e_sb")
            s_col = statsp.tile([128, 1], F32, tag="s_col")
            nc.scalar.activation(
                out=e_sb,
                in_=mstat,
                func=EXP,
                bias=ngb_sb,
                scale=1.0,
                accum_out=s_col,
            )
            tot_ps = ps_misc.tile([1, 1], F32, tag="misc")
            nc.tensor.matmul(tot_ps, s_col, ones_c, start=True, stop=True)
            rt_sb = statsp.tile([1, 1], F32, tag="rt")
            nc.vector.reciprocal(out=rt_sb, in_=tot_ps)
            q2c_ps = ps_misc.tile([1, E], F32, tag="misc")
            for t in range(NT):
                nc.tensor.matmul(
                    q2c_ps,
                    e_sb[:, t : t + 1],
                    xcbfs[t // 4][:, t % 4, :],
                    start=(t == 0),
                    stop=(t == NT - 1),
                )
            q2c_sb = statsp.tile([1, E], F32, tag="q2c_sb")
            nc.scalar.activation(out=q2c_sb, in_=q2c_ps, func=CPY, scale=rt_sb)
            q2cb_ps = ps_misc.tile([128, E], F32, tag="misc")
            nc.tensor.matmul(q2cb_ps, ones_r, q2c_sb, start=True, stop=True)
            q2cb_sb = statsp.tile([128, E], F32, tag="q2cb")
            nc.vector.tensor_copy(out=q2cb_sb, in_=q2cb_ps)
            # ---- ctx * q2c + stream out cols 3E:4E -----------------------
            for g in range(NG):
                stg = stgs[b][g]
                for lane in range(4):
                    nc.vector.tensor_mul(
                        stg[:, lane, 3 * E : 4 * E],
                        stg[:, lane, 0:E],
                        q2cb_sb,
                    )
                for h in range(2):
                    r0 = g * 512 + h * 256
                    nc.sync.dma_start(
                        out=out_ext[
                            b, r0 : r0 + 256, 3 * E : 4 * E
                        ].rearrange("(t p) f -> p t f", p=128),
                        in_=stg[:, 2 * h : 2 * h + 2, 3 * E : 4 * E],
                    )


_NC_CACHE = None


def _build():
    global _NC_CACHE
    if _NC_CACHE is not None:
        return _NC_CACHE
    nc = bacc.Bacc(
        "TRN2", target_bir_lowering=False, debug=False, num_devices=NCORES
    )
    ctx_in = nc.dram_tensor("context", [BPC, C, E], F32, kind="ExternalInput").ap()
    q_in = nc.dram_tensor("question", [BPC, Q, E], F32, kind="ExternalInput").ap()
    wq_in = nc.dram_tensor("w_question", [E], F32, kind="ExternalInput").ap()
    wc_in = nc.dram_tensor("w_context", [E], F32, kind="ExternalInput").ap()
    wm_in = nc.dram_tensor("w_multiple", [E], F32, kind="ExternalInput").ap()
    out_ext = nc.dram_tensor("out", [BPC, C, 4 * E], F32, kind="ExternalOutput").ap()
    with tile.TileContext(nc) as tc:
        _body(tc, out_ext, ctx_in, q_in, wq_in, wc_in, wm_in)
    nc.compile()
    _NC_CACHE = nc
    return nc


def _run(inputs, trace=False, **kw):
    nc = _build()
    context = np.ascontiguousarray(np.asarray(inputs["context"], dtype=np.float32))
    question = np.ascontiguousarray(np.asarray(inputs["question"], dtype=np.float32))
    wq = np.ascontiguousarray(np.asarray(inputs["w_question"], dtype=np.float32))
    wc = np.ascontiguousarray(np.asarray(inputs["w_context"], dtype=np.float32))
    wm = np.ascontiguousarray(np.asarray(inputs["w_multiple"], dtype=np.float32))
    in_maps = []
    for i in range(NCORES):
        sl = slice(i * BPC, (i + 1) * BPC)
        in_maps.append(
            {
                "context": context[sl],
                "question": question[sl],
                "w_question": wq,
                "w_context": wc,
                "w_multiple": wm,
            }
        )
    res = run_bass_kernel_spmd(
        nc, in_maps, core_ids=list(range(NCORES)), trace=trace, **kw
    )
    out = np.concatenate([res.results[i]["out"] for i in range(NCORES)], axis=0)
    return out, res


def kernel(**inputs):
    try:
        out, _ = _run(inputs, trace=False)
    except Exception:
        # transient device errors (e.g. a wedged core from a prior run)
        # usually clear on retry
        out, _ = _run(inputs, trace=False)
    return out
